# revision 21
# baseline (speedup 1.0000x reference)
"""Trainium2 Bass kernel for nn_CascadedAttention (B=8, T=128, D=512, O=512).

Strategy: data-parallel over batch across 8 NeuronCores (1 batch element
per core), weights replicated. The scan recurrence runs fully on-device,
fully unrolled, with column-major (O-on-partitions) state layout.

Self-contained: hardcodes all shapes; only imports the installed
concourse (bass) stack.
"""

import sys

for _p in ("/opt/trn_rl_repo", "/root/.axon_site/_ro/trn_rl_repo"):
    if _p not in sys.path:
        sys.path.append(_p)

import numpy as np

import concourse.bass as bass
import concourse.bacc as bacc
import concourse.mybir as mybir
from concourse import tile
from concourse.bass_utils import run_bass_kernel_spmd

B, T, D, O = 8, 128, 512, 512
OT = O // 128  # 4 o-tiles
DT = D // 128  # 4 d-tiles
FP32 = mybir.dt.float32
AF = mybir.ActivationFunctionType
ALU = mybir.AluOpType

# dtype knobs (bf16 halves PE weight-load time via FWL; fp32 = exact)
WA_BF16 = False
E_BF16 = False
ICO_BF16 = False
# assembly (u'/Z + IUoB) via one STT divide; False = reciprocal + STT mult
# (divide fails walrus ISA check NCC_IXCG864 -> keep False)
USE_DIVIDE = False


def build_nc():
    # Bacc (not raw Bass): its compile() legalizes sync waits for walrus
    # (TRN2 allows at most one wait per instruction).
    nc = bacc.Bacc(None, target_bir_lowering=False, debug=False)

    x_d = nc.declare_dram_parameter("x", [T, D], FP32, isOutput=False)
    Wa_d = nc.declare_dram_parameter("Wa", [O, O], FP32, isOutput=False)
    Ua_d = nc.declare_dram_parameter("Ua", [D, O], FP32, isOutput=False)
    Uo_d = nc.declare_dram_parameter("Uo", [D, O], FP32, isOutput=False)
    Co_d = nc.declare_dram_parameter("Co", [D, O], FP32, isOutput=False)
    Va_d = nc.declare_dram_parameter("Va_col", [128, OT], FP32, isOutput=False)
    Ba_d = nc.declare_dram_parameter("Ba_col", [128, OT], FP32, isOutput=False)
    Bo_d = nc.declare_dram_parameter("Bo_col", [128, OT], FP32, isOutput=False)
    oeb_d = nc.declare_dram_parameter("oeb", [128, 2 * OT], FP32, isOutput=False)
    Id_d = nc.declare_dram_parameter("Id", [128, 128], FP32, isOutput=False)
    out_d = nc.declare_dram_parameter("out", [T, O], FP32, isOutput=True)

    wdt = mybir.dt.bfloat16 if WA_BF16 else FP32
    edt = mybir.dt.bfloat16 if E_BF16 else FP32
    cdt = mybir.dt.bfloat16 if ICO_BF16 else FP32

    with tile.TileContext(nc) as tc:
        with (
            tc.tile_pool(name="persist", bufs=1) as pp,
            tc.tile_pool(name="wpool", bufs=1) as wp,
        ):
            # ---- persistent SBUF tensors ----
            x_sb = pp.tile([128, D], FP32, tag="x")          # [tau, d]
            xT_sb = pp.tile([128, T * DT], FP32, tag="xT")   # tile dt at cols dt*128+tau
            Wa_sb = wp.tile([128, O * OT], wdt, tag="Wa")    # [o'', ot*O + o']
            Ua_sb = wp.tile([128, O * DT], FP32, tag="Ua")
            Uo_sb = wp.tile([128, O * DT], FP32, tag="Uo")
            Co_sb = wp.tile([128, O * DT], FP32, tag="Co")
            Va_sb = pp.tile([128, OT], edt, tag="Va")
            Ba_sb = pp.tile([128, OT], FP32, tag="Ba")
            Bo_sb = pp.tile([128, OT], FP32, tag="Bo")
            oeb_sb = pp.tile([128, 2 * OT], FP32, tag="oeb")  # [1|embWo] pairs
            Id_sb = pp.tile([128, 128], FP32, tag="Id")
            UaH_sb = pp.tile([128, T * OT], FP32, tag="UaH")   # [o'', ot*T+tau]
            IUoB_sb = pp.tile([128, T * OT], FP32, tag="IUoB")  # [o'', tau*OT+kt]
            ICo_sb = pp.tile([128, O], cdt, tag="ICo")          # [tau, o]
            ones128 = pp.tile([128, 128], cdt, tag="ones128")
            ones_row = pp.tile([1, 128], FP32, tag="ones_r")
            zstate = pp.tile([128, OT], FP32, tag="zstate")
            zhw = pp.tile([128, 1], FP32, tag="zhw")
            out_sb = pp.tile([128, T * OT], FP32, tag="outb")  # [o'', t*OT+kt]
            woyrow = pp.tile([1, T], FP32, tag="woyrow")

            # ---- DMA in ----
            nc.sync.dma_start(x_sb[:, :], x_d[:, :])
            for dt in range(DT):
                nc.sync.dma_start(
                    Ua_sb[:, dt * O:(dt + 1) * O], Ua_d[dt * 128:(dt + 1) * 128, :]
                )
            for dt in range(DT):
                nc.sync.dma_start(
                    Uo_sb[:, dt * O:(dt + 1) * O], Uo_d[dt * 128:(dt + 1) * 128, :]
                )
            for dt in range(DT):
                nc.sync.dma_start(
                    Co_sb[:, dt * O:(dt + 1) * O], Co_d[dt * 128:(dt + 1) * 128, :]
                )
            if WA_BF16:
                WaF_sb = pp.tile([128, O * OT], FP32, tag="WaF")
                for ot in range(OT):
                    nc.sync.dma_start(
                        WaF_sb[:, ot * O:(ot + 1) * O], Wa_d[ot * 128:(ot + 1) * 128, :]
                    )
                for ot in range(OT):
                    nc.vector.tensor_copy(
                        Wa_sb[:, ot * O:(ot + 1) * O], WaF_sb[:, ot * O:(ot + 1) * O]
                    )
            else:
                for ot in range(OT):
                    nc.sync.dma_start(
                        Wa_sb[:, ot * O:(ot + 1) * O], Wa_d[ot * 128:(ot + 1) * 128, :]
                    )
            if E_BF16:
                VaF_sb = pp.tile([128, OT], FP32, tag="VaF")
                nc.sync.dma_start(VaF_sb[:, :], Va_d[:, :])
                nc.vector.tensor_copy(Va_sb[:, :], VaF_sb[:, :])
            else:
                nc.sync.dma_start(Va_sb[:, :], Va_d[:, :])
            nc.sync.dma_start(Ba_sb[:, :], Ba_d[:, :])
            nc.sync.dma_start(Bo_sb[:, :], Bo_d[:, :])
            nc.sync.dma_start(oeb_sb[:, :], oeb_d[:, :])
            nc.sync.dma_start(Id_sb[:, :], Id_d[:, :])

            # ---- constants ----
            nc.vector.memset(ones128[:, :], 1.0)
            nc.vector.memset(ones_row[:, :], 1.0)
            nc.vector.memset(zstate[:, :], 0.0)
            nc.vector.memset(zhw[:, :], 0.0)

            # collapse the DMA-queue fan-in: downstream instructions wait on
            # one barrier instead of many DMA semaphores (walrus ISA structs
            # have few sync-wait slots)
            tc.strict_bb_all_engine_barrier()

            # ---- precompute ----
            with tc.tile_pool(name="pre_ps", bufs=2, space="PSUM") as prep:
                # xT: transpose x tiles
                for dt in range(DT):
                    pt = prep.tile([128, 128], FP32, tag="pt")
                    nc.tensor.transpose(
                        pt[:, :], x_sb[:, dt * 128:(dt + 1) * 128], Id_sb[:, :]
                    )
                    nc.vector.tensor_copy(xT_sb[:, dt * 128:(dt + 1) * 128], pt[:, :])
                # UaH_T[o'', ot*T+tau] = sum_d Ua[d, o] * x[tau, d]  (+Ba via bias)
                for ot in range(OT):
                    pu = prep.tile([128, 128], FP32, tag="pu")
                    for dt in range(DT):
                        nc.tensor.matmul(
                            pu[:, :],
                            Ua_sb[:, dt * O + ot * 128: dt * O + (ot + 1) * 128],
                            xT_sb[:, dt * 128:(dt + 1) * 128],
                            start=(dt == 0),
                            stop=(dt == DT - 1),
                        )
                    nc.scalar.activation(
                        UaH_sb[:, ot * T:(ot + 1) * T], pu[:, :], AF.Identity,
                        bias=Ba_sb[:, ot:ot + 1],
                    )
                # IUoB[o'', tau*OT+kt] = x[tau]@Uo + Bo
                for ot in range(OT):
                    pi = prep.tile([128, 128], FP32, tag="pu")
                    for dt in range(DT):
                        nc.tensor.matmul(
                            pi[:, :],
                            Uo_sb[:, dt * O + ot * 128: dt * O + (ot + 1) * 128],
                            xT_sb[:, dt * 128:(dt + 1) * 128],
                            start=(dt == 0),
                            stop=(dt == DT - 1),
                        )
                    dst = IUoB_sb[:, ot:ot + (T - 1) * OT + 1:OT]
                    nc.scalar.activation(
                        dst, pi[:, :], AF.Identity, bias=Bo_sb[:, ot:ot + 1]
                    )
                # ICo[tau, o] = x[tau] @ Co
                pc = prep.tile([128, O], FP32, tag="pc")
                for dt in range(DT):
                    nc.tensor.matmul(
                        pc[:, :],
                        xT_sb[:, dt * 128:(dt + 1) * 128],
                        Co_sb[:, dt * O:(dt + 1) * O],
                        start=(dt == 0),
                        stop=(dt == DT - 1),
                    )
                nc.vector.tensor_copy(ICo_sb[:, :], pc[:, :])

            # ---- the scan ----
            with (
                tc.tile_pool(name="sb_loop", bufs=3) as lp,
                tc.tile_pool(name="e_pool", bufs=2) as ep_pool,
                tc.tile_pool(name="was_ps", bufs=1, space="PSUM") as wasp,
                tc.tile_pool(name="pred_ps", bufs=2, space="PSUM") as predp,
                tc.tile_pool(name="sc_ps", bufs=1, space="PSUM") as scp,
                tc.tile_pool(name="misc_ps", bufs=2, space="PSUM") as miscp,
                tc.tile_pool(name="zb_ps", bufs=1, space="PSUM") as zbp,
            ):
                hwb_prev = zhw
                for t in range(T):
                    state = zstate if t == 0 else out_sb[:, (t - 1) * OT: t * OT]

                    # th = tanh(pred/2 + WoY/2)  (sigmoid affine folded into
                    # Ba_adj = Ba + 0.5*colsum(Wa) on the host + 0.5 scale on
                    # the WaS psum copy)
                    th = lp.tile([128, OT], wdt, tag="th")
                    nc.scalar.activation(
                        th[:, :], state, AF.Tanh,
                        bias=hwb_prev[:, 0:1], scale=0.5,
                    )
                    # ep = exp(pred_noWoY)  (softmax(pred) invariant to +WoY)
                    ep = lp.tile([128, OT], FP32, tag="ep")
                    nc.scalar.activation(ep[:, :], state, AF.Exp)

                    # W1[o'] = sum_o th[o]*Wa[o,o']  -> column layout [128, OT]
                    # PE order: 16 WaS mms, then scores; the 0.5-scaled psum
                    # copies run on ACT (same engine as tanh -> no extra sem).
                    w_ps = [
                        wasp.tile([128, 2], FP32, tag=f"wps{i}", name=f"wps{i}_{t}")
                        for i in range(2)
                    ]
                    WaS_sb = lp.tile([128, OT], FP32, tag="WaS")
                    e_sb = ep_pool.tile([128, O], edt, tag="e")
                    sc = scp.tile([128, 1], FP32, tag="sc")
                    for opt in range(OT):
                        ph = w_ps[opt // 2]
                        col = opt % 2
                        for ot in range(OT):
                            nc.tensor.matmul(
                                ph[:, col:col + 1],
                                Wa_sb[:, ot * O + opt * 128: ot * O + (opt + 1) * 128],
                                th[:, ot:ot + 1],
                                start=(ot == 0),
                                stop=(ot == OT - 1),
                            )
                        if col == 1:
                            pair = opt - 1
                            # 0.5*W1 psum->sbuf on DVE (idle engine; keeps the
                            # ACT tanh ladder uninterrupted)
                            nc.vector.tensor_scalar_mul(
                                WaS_sb[:, pair:pair + 2], ph[:, 0:2], 0.5
                            )
                            for q in (pair, pair + 1):
                                # e_T tile = tanh(UaH_T + Ba_adj + 0.5*W1)
                                nc.scalar.activation(
                                    e_sb[:, q * T:(q + 1) * T],
                                    UaH_sb[:, q * T:(q + 1) * T],
                                    AF.Tanh, bias=WaS_sb[:, q:q + 1],
                                )
                    for q in range(OT):
                        # scores[tau] += Va[o'] . e_T[o', tau]
                        nc.tensor.matmul(
                            sc[:, 0:1],
                            e_sb[:, q * T:(q + 1) * T],
                            Va_sb[:, q:q + 1],
                            start=(q == 0),
                            stop=(q == OT - 1),
                        )

                    # WoY numerators: [Z2, numerW] = sum_o ep[o] * [1, embWo[o]]
                    misc = miscp.tile([128, 8], FP32, tag="misc")
                    for kt in range(OT):
                        nc.tensor.matmul(
                            misc[0:1, 1:3],
                            ep[:, kt:kt + 1],
                            oeb_sb[:, 2 * kt:2 * kt + 2],
                            start=(kt == 0),
                            stop=(kt == OT - 1),
                        )
                    # WoY_t = numerW / Z2 (all off the critical chain)
                    rz2 = lp.tile([1, 1], FP32, tag="rz2")
                    nc.vector.reciprocal(rz2[:, :], misc[0:1, 1:2])
                    nc.vector.tensor_mul(woyrow[:, t:t + 1], misc[0:1, 2:3], rz2[:, :])
                    hwrow = lp.tile([1, 1], FP32, tag="hwrow")
                    nc.vector.tensor_scalar_mul(hwrow[:, :], woyrow[:, t:t + 1], 0.5)
                    # broadcast halfWoY to all partitions (bias for next tanh)
                    nc.tensor.matmul(
                        misc[:, 3:4], ones_row[:, :], hwrow[:, :],
                        start=True, stop=True,
                    )
                    hwb = lp.tile([128, 1], FP32, tag="hwb")
                    nc.vector.tensor_copy(hwb[:, :], misc[:, 3:4])

                    # softmax over tau (no max-sub: scores are O(0.3))
                    eh = lp.tile([128, 1], cdt, tag="eh")
                    nc.scalar.activation(eh[:, :], sc[:, :], AF.Exp)
                    # Z broadcast to all partitions in one mm: ones128.T @ eh
                    zb = zbp.tile([128, 1], FP32, tag="zb")
                    nc.tensor.matmul(
                        zb[:, :], ones128[:, :], eh[:, :], start=True, stop=True
                    )

                    # u'[o] = sum_tau eh[tau] * ICo[tau, o] (unnormalized)
                    pred = predp.tile([128, OT], FP32, tag="pred")
                    for m in range(OT):
                        nc.tensor.matmul(
                            pred[:, m:m + 1],
                            ICo_sb[:, m * 128:(m + 1) * 128],
                            eh[:, :],
                            start=True, stop=True,
                        )
                    # pred_noWoY_t = u'/Z + (IUo[t-1] + Bo)
                    tm1 = (t - 1) % T
                    if USE_DIVIDE:
                        nc.vector.scalar_tensor_tensor(
                            out_sb[:, t * OT:(t + 1) * OT],
                            pred[:, :],
                            zb[:, 0:1],
                            IUoB_sb[:, tm1 * OT:(tm1 + 1) * OT],
                            ALU.divide, ALU.add,
                        )
                    else:
                        rz = lp.tile([128, 1], FP32, tag="rz")
                        nc.vector.reciprocal(rz[:, :], zb[:, :])
                        nc.vector.scalar_tensor_tensor(
                            out_sb[:, t * OT:(t + 1) * OT],
                            pred[:, :],
                            rz[:, 0:1],
                            IUoB_sb[:, tm1 * OT:(tm1 + 1) * OT],
                            ALU.mult, ALU.add,
                        )
                    hwb_prev = hwb

            # ---- epilogue: add WoY (over t) and transpose to [tau, o] ----
            with (
                tc.tile_pool(name="ep_ps", bufs=2, space="PSUM") as epp,
                tc.tile_pool(name="ep_sb", bufs=2) as eps,
            ):
                outT = pp.tile([128, O], FP32, tag="outT")
                for kt in range(OT):
                    po = epp.tile([128, 128], FP32, tag="po")
                    nc.tensor.transpose(
                        po[:, :],
                        out_sb[:, kt:kt + (T - 1) * OT + 1:OT],
                        Id_sb[:, :],
                    )
                    # += WoY[tau] broadcast along o''
                    nc.tensor.matmul(
                        po[:, :], woyrow[:, :], ones_row[:, :],
                        start=False, stop=True, skip_group_check=True,
                    )
                    nc.vector.tensor_copy(outT[:, kt * 128:(kt + 1) * 128], po[:, :])
                nc.sync.dma_start(out_d[:, :], outT[:, :])

    nc.compile()
    return nc


_NC_CACHE = {}


def _get_nc():
    if "nc" not in _NC_CACHE:
        _NC_CACHE["nc"] = build_nc()
    return _NC_CACHE["nc"]


def make_in_maps(inputs, Wa, Ua, Va, Ba, Wo, Uo, Co, Bo, emb):
    Wa = np.asarray(Wa, np.float32)
    Ua = np.asarray(Ua, np.float32)
    Uo = np.asarray(Uo, np.float32)
    Co = np.asarray(Co, np.float32)
    Va_col = np.ascontiguousarray(
        np.asarray(Va, np.float32)[:, 0].reshape(OT, 128).T
    )
    # fold sigmoid's affine (s = 0.5*tanh + 0.5) into the attention key bias:
    # WaS = s@Wa = 0.5*(tanh_h@Wa) + 0.5*colsum(Wa)
    ba_adj = (
        np.asarray(Ba, np.float64)[0]
        + 0.5 * np.asarray(Wa, np.float64).sum(axis=0)
    ).astype(np.float32)
    Ba_col = np.ascontiguousarray(ba_adj.reshape(OT, 128).T)
    Bo_col = np.ascontiguousarray(
        np.asarray(Bo, np.float32)[0].reshape(OT, 128).T
    )
    ebW = (np.asarray(emb, np.float64) @ np.asarray(Wo, np.float64)).astype(np.float32)
    ebW_col = ebW[:, 0].reshape(OT, 128).T
    oeb = np.ones((128, 2 * OT), dtype=np.float32)
    oeb[:, 1::2] = ebW_col
    oeb = np.ascontiguousarray(oeb)
    Id = np.eye(128, dtype=np.float32)
    shared = dict(
        Wa=Wa, Ua=Ua, Uo=Uo, Co=Co,
        Va_col=Va_col, Ba_col=Ba_col, Bo_col=Bo_col, oeb=oeb, Id=Id,
    )
    return [
        {"x": np.ascontiguousarray(np.asarray(inputs[b], np.float32)), **shared}
        for b in range(B)
    ]


def kernel(inputs, Wa, Ua, Va, Ba, Wo, Uo, Co, Bo, emb):
    nc = _get_nc()
    in_maps = make_in_maps(inputs, Wa, Ua, Va, Ba, Wo, Uo, Co, Bo, emb)
    res = run_bass_kernel_spmd(nc, in_maps, list(range(B)))
    out = np.stack([res.results[b]["out"] for b in range(B)], axis=0)
    return out.astype(np.float32)


if __name__ == "__main__":
    rng = np.random.default_rng(0)
    w = 0.02
    ins = dict(
        inputs=rng.standard_normal((B, T, D), dtype=np.float32),
        Wa=rng.standard_normal((O, O), dtype=np.float32) * w,
        Ua=rng.standard_normal((D, O), dtype=np.float32) * w,
        Va=rng.standard_normal((O, 1), dtype=np.float32) * w,
        Ba=rng.standard_normal((1, O), dtype=np.float32) * w,
        Wo=rng.standard_normal((O, 1), dtype=np.float32) * w,
        Uo=rng.standard_normal((D, O), dtype=np.float32) * w,
        Co=rng.standard_normal((D, O), dtype=np.float32) * w,
        Bo=rng.standard_normal((1, O), dtype=np.float32) * w,
        emb=rng.standard_normal((O, O), dtype=np.float32) * w,
    )
    out = kernel(**ins)
    print(out.shape, out.dtype, np.abs(out).mean())


# revision 22
# speedup vs baseline: 2.5035x; 2.5035x over previous
"""Trainium2 Bass kernel for nn_CascadedAttention (B=8, T=128, D=512, O=512).

Strategy: data-parallel over batch across 8 NeuronCores (1 batch element
per core), weights replicated. The scan recurrence runs fully on-device,
fully unrolled, with column-major (O-on-partitions) state layout.

Self-contained: hardcodes all shapes; only imports the installed
concourse (bass) stack.
"""

import sys

for _p in ("/opt/trn_rl_repo", "/root/.axon_site/_ro/trn_rl_repo"):
    if _p not in sys.path:
        sys.path.append(_p)

import numpy as np

import concourse.bass as bass
import concourse.bacc as bacc
import concourse.mybir as mybir
from concourse import tile
from concourse.bass_utils import run_bass_kernel_spmd

B, T, D, O = 8, 128, 512, 512
OT = O // 128  # 4 o-tiles
DT = D // 128  # 4 d-tiles
FP32 = mybir.dt.float32
AF = mybir.ActivationFunctionType
ALU = mybir.AluOpType

# dtype knobs (bf16 halves PE weight-load time via FWL; fp32 = exact)
WA_BF16 = False
E_BF16 = False
ICO_BF16 = False
# assembly (u'/Z + IUoB) via one STT divide; False = reciprocal + STT mult
# (divide fails walrus ISA check NCC_IXCG864 -> keep False)
USE_DIVIDE = False


def build_nc():
    # Bacc (not raw Bass): its compile() legalizes sync waits for walrus
    # (TRN2 allows at most one wait per instruction).
    nc = bacc.Bacc(None, target_bir_lowering=False, debug=False)

    x_d = nc.declare_dram_parameter("x", [T, D], FP32, isOutput=False)
    Wa_d = nc.declare_dram_parameter("Wa", [O, O], FP32, isOutput=False)
    Ua_d = nc.declare_dram_parameter("Ua", [D, O], FP32, isOutput=False)
    Uo_d = nc.declare_dram_parameter("Uo", [D, O], FP32, isOutput=False)
    Co_d = nc.declare_dram_parameter("Co", [D, O], FP32, isOutput=False)
    Va_d = nc.declare_dram_parameter("Va_col", [128, OT], FP32, isOutput=False)
    Ba_d = nc.declare_dram_parameter("Ba_col", [128, OT], FP32, isOutput=False)
    Bo_d = nc.declare_dram_parameter("Bo_col", [128, OT], FP32, isOutput=False)
    oeb_d = nc.declare_dram_parameter("oeb", [128, 2 * OT], FP32, isOutput=False)
    Id_d = nc.declare_dram_parameter("Id", [128, 128], FP32, isOutput=False)
    out_d = nc.declare_dram_parameter("out", [T, O], FP32, isOutput=True)

    wdt = mybir.dt.bfloat16 if WA_BF16 else FP32
    edt = mybir.dt.bfloat16 if E_BF16 else FP32
    cdt = mybir.dt.bfloat16 if ICO_BF16 else FP32

    with tile.TileContext(nc) as tc:
        with (
            tc.tile_pool(name="persist", bufs=1) as pp,
            tc.tile_pool(name="wpool", bufs=1) as wp,
        ):
            # ---- persistent SBUF tensors ----
            x_sb = pp.tile([128, D], FP32, tag="x")          # [tau, d]
            xT_sb = pp.tile([128, T * DT], FP32, tag="xT")   # tile dt at cols dt*128+tau
            Wa_sb = wp.tile([128, O * OT], wdt, tag="Wa")    # [o'', ot*O + o']
            Ua_sb = wp.tile([128, O * DT], FP32, tag="Ua")
            Uo_sb = wp.tile([128, O * DT], FP32, tag="Uo")
            Co_sb = wp.tile([128, O * DT], FP32, tag="Co")
            Va_sb = pp.tile([128, OT], edt, tag="Va")
            Ba_sb = pp.tile([128, OT], FP32, tag="Ba")
            Bo_sb = pp.tile([128, OT], FP32, tag="Bo")
            oeb_sb = pp.tile([128, 2 * OT], FP32, tag="oeb")  # [1|embWo] pairs
            Id_sb = pp.tile([128, 128], FP32, tag="Id")
            UaH_sb = pp.tile([128, T * OT], FP32, tag="UaH")   # [o'', ot*T+tau]
            IUoB_sb = pp.tile([128, T * OT], FP32, tag="IUoB")  # [o'', tau*OT+kt]
            ICo_sb = pp.tile([128, O], cdt, tag="ICo")          # [tau, o]
            ones128 = pp.tile([128, 128], cdt, tag="ones128")
            ones_row = pp.tile([1, 128], FP32, tag="ones_r")
            zstate = pp.tile([128, OT], FP32, tag="zstate")
            zhw = pp.tile([128, 1], FP32, tag="zhw")
            out_sb = pp.tile([128, T * OT], FP32, tag="outb")  # [o'', t*OT+kt]
            woyrow = pp.tile([1, T], FP32, tag="woyrow")

            # ---- DMA in ----
            nc.sync.dma_start(x_sb[:, :], x_d[:, :])
            for dt in range(DT):
                nc.sync.dma_start(
                    Ua_sb[:, dt * O:(dt + 1) * O], Ua_d[dt * 128:(dt + 1) * 128, :]
                )
            for dt in range(DT):
                nc.sync.dma_start(
                    Uo_sb[:, dt * O:(dt + 1) * O], Uo_d[dt * 128:(dt + 1) * 128, :]
                )
            for dt in range(DT):
                nc.sync.dma_start(
                    Co_sb[:, dt * O:(dt + 1) * O], Co_d[dt * 128:(dt + 1) * 128, :]
                )
            if WA_BF16:
                WaF_sb = pp.tile([128, O * OT], FP32, tag="WaF")
                for ot in range(OT):
                    nc.sync.dma_start(
                        WaF_sb[:, ot * O:(ot + 1) * O], Wa_d[ot * 128:(ot + 1) * 128, :]
                    )
                for ot in range(OT):
                    nc.vector.tensor_copy(
                        Wa_sb[:, ot * O:(ot + 1) * O], WaF_sb[:, ot * O:(ot + 1) * O]
                    )
            else:
                for ot in range(OT):
                    nc.sync.dma_start(
                        Wa_sb[:, ot * O:(ot + 1) * O], Wa_d[ot * 128:(ot + 1) * 128, :]
                    )
            if E_BF16:
                VaF_sb = pp.tile([128, OT], FP32, tag="VaF")
                nc.sync.dma_start(VaF_sb[:, :], Va_d[:, :])
                nc.vector.tensor_copy(Va_sb[:, :], VaF_sb[:, :])
            else:
                nc.sync.dma_start(Va_sb[:, :], Va_d[:, :])
            nc.sync.dma_start(Ba_sb[:, :], Ba_d[:, :])
            nc.sync.dma_start(Bo_sb[:, :], Bo_d[:, :])
            nc.sync.dma_start(oeb_sb[:, :], oeb_d[:, :])
            nc.sync.dma_start(Id_sb[:, :], Id_d[:, :])

            # ---- constants ----
            nc.vector.memset(ones128[:, :], 1.0)
            nc.vector.memset(ones_row[:, :], 1.0)
            nc.vector.memset(zstate[:, :], 0.0)
            nc.vector.memset(zhw[:, :], 0.0)

            # collapse the DMA-queue fan-in: downstream instructions wait on
            # one barrier instead of many DMA semaphores (walrus ISA structs
            # have few sync-wait slots)
            tc.strict_bb_all_engine_barrier()

            # ---- precompute ----
            with tc.tile_pool(name="pre_ps", bufs=2, space="PSUM") as prep:
                # xT: transpose x tiles
                for dt in range(DT):
                    pt = prep.tile([128, 128], FP32, tag="pt")
                    nc.tensor.transpose(
                        pt[:, :], x_sb[:, dt * 128:(dt + 1) * 128], Id_sb[:, :]
                    )
                    nc.vector.tensor_copy(xT_sb[:, dt * 128:(dt + 1) * 128], pt[:, :])
                # UaH_T[o'', ot*T+tau] = sum_d Ua[d, o] * x[tau, d]  (+Ba via bias)
                for ot in range(OT):
                    pu = prep.tile([128, 128], FP32, tag="pu")
                    for dt in range(DT):
                        nc.tensor.matmul(
                            pu[:, :],
                            Ua_sb[:, dt * O + ot * 128: dt * O + (ot + 1) * 128],
                            xT_sb[:, dt * 128:(dt + 1) * 128],
                            start=(dt == 0),
                            stop=(dt == DT - 1),
                        )
                    nc.scalar.activation(
                        UaH_sb[:, ot * T:(ot + 1) * T], pu[:, :], AF.Identity,
                        bias=Ba_sb[:, ot:ot + 1],
                    )
                # IUoB[o'', tau*OT+kt] = x[tau]@Uo + Bo
                for ot in range(OT):
                    pi = prep.tile([128, 128], FP32, tag="pu")
                    for dt in range(DT):
                        nc.tensor.matmul(
                            pi[:, :],
                            Uo_sb[:, dt * O + ot * 128: dt * O + (ot + 1) * 128],
                            xT_sb[:, dt * 128:(dt + 1) * 128],
                            start=(dt == 0),
                            stop=(dt == DT - 1),
                        )
                    dst = IUoB_sb[:, ot:ot + (T - 1) * OT + 1:OT]
                    nc.scalar.activation(
                        dst, pi[:, :], AF.Identity, bias=Bo_sb[:, ot:ot + 1]
                    )
                # ICo[tau, o] = x[tau] @ Co
                pc = prep.tile([128, O], FP32, tag="pc")
                for dt in range(DT):
                    nc.tensor.matmul(
                        pc[:, :],
                        xT_sb[:, dt * 128:(dt + 1) * 128],
                        Co_sb[:, dt * O:(dt + 1) * O],
                        start=(dt == 0),
                        stop=(dt == DT - 1),
                    )
                nc.vector.tensor_copy(ICo_sb[:, :], pc[:, :])

            # ---- the scan ----
            with (
                tc.tile_pool(name="sb_loop", bufs=3) as lp,
                tc.tile_pool(name="e_pool", bufs=2) as ep_pool,
                tc.tile_pool(name="was_ps", bufs=1, space="PSUM") as wasp,
                tc.tile_pool(name="pred_ps", bufs=2, space="PSUM") as predp,
                tc.tile_pool(name="sc_ps", bufs=1, space="PSUM") as scp,
                tc.tile_pool(name="misc_ps", bufs=2, space="PSUM") as miscp,
                tc.tile_pool(name="zb_ps", bufs=1, space="PSUM") as zbp,
            ):
                hwb_prev = zhw
                for t in range(T):
                    state = zstate if t == 0 else out_sb[:, (t - 1) * OT: t * OT]

                    # th = tanh(pred/2 + WoY/2)  (sigmoid affine folded into
                    # Ba_adj = Ba + 0.5*colsum(Wa) on the host + 0.5 scale on
                    # the WaS psum copy)
                    th = lp.tile([128, OT], wdt, tag="th")
                    nc.scalar.activation(
                        th[:, :], state, AF.Tanh,
                        bias=hwb_prev[:, 0:1], scale=0.5,
                    )
                    # ep = exp(pred_noWoY)  (softmax(pred) invariant to +WoY)
                    ep = lp.tile([128, OT], FP32, tag="ep")
                    nc.scalar.activation(ep[:, :], state, AF.Exp)

                    # W1[o'] = sum_o th[o]*Wa[o,o']  -> column layout [128, OT]
                    # PE order: 16 WaS mms, then scores; the 0.5-scaled psum
                    # copies run on ACT (same engine as tanh -> no extra sem).
                    w_ps = [
                        wasp.tile([128, 2], FP32, tag=f"wps{i}", name=f"wps{i}_{t}")
                        for i in range(2)
                    ]
                    WaS_sb = lp.tile([128, OT], FP32, tag="WaS")
                    e_sb = ep_pool.tile([128, O], edt, tag="e")
                    sc = scp.tile([128, 1], FP32, tag="sc")
                    for opt in range(OT):
                        ph = w_ps[opt // 2]
                        col = opt % 2
                        for ot in range(OT):
                            nc.tensor.matmul(
                                ph[:, col:col + 1],
                                Wa_sb[:, ot * O + opt * 128: ot * O + (opt + 1) * 128],
                                th[:, ot:ot + 1],
                                start=(ot == 0),
                                stop=(ot == OT - 1),
                            )
                        if col == 1:
                            pair = opt - 1
                            # 0.5*W1 psum->sbuf on ACT: same engine as the
                            # tanh ladder -> no cross-engine sem before tanh0
                            nc.scalar.mul(WaS_sb[:, pair:pair + 2], ph[:, 0:2], 0.5)
                            for q in (pair, pair + 1):
                                # e_T tile = tanh(UaH_T + Ba_adj + 0.5*W1)
                                nc.scalar.activation(
                                    e_sb[:, q * T:(q + 1) * T],
                                    UaH_sb[:, q * T:(q + 1) * T],
                                    AF.Tanh, bias=WaS_sb[:, q:q + 1],
                                )
                    for q in range(OT):
                        # scores[tau] += Va[o'] . e_T[o', tau]
                        nc.tensor.matmul(
                            sc[:, 0:1],
                            e_sb[:, q * T:(q + 1) * T],
                            Va_sb[:, q:q + 1],
                            start=(q == 0),
                            stop=(q == OT - 1),
                        )

                    # WoY numerators: [Z2, numerW] = sum_o ep[o] * [1, embWo[o]]
                    misc = miscp.tile([128, 8], FP32, tag="misc")
                    for kt in range(OT):
                        nc.tensor.matmul(
                            misc[0:1, 1:3],
                            ep[:, kt:kt + 1],
                            oeb_sb[:, 2 * kt:2 * kt + 2],
                            start=(kt == 0),
                            stop=(kt == OT - 1),
                        )
                    # WoY_t = numerW / Z2 (all off the critical chain)
                    rz2 = lp.tile([1, 1], FP32, tag="rz2")
                    nc.vector.reciprocal(rz2[:, :], misc[0:1, 1:2])
                    nc.vector.tensor_mul(woyrow[:, t:t + 1], misc[0:1, 2:3], rz2[:, :])
                    hwrow = lp.tile([1, 1], FP32, tag="hwrow")
                    nc.vector.tensor_scalar_mul(hwrow[:, :], woyrow[:, t:t + 1], 0.5)
                    # broadcast halfWoY to all partitions (bias for next tanh)
                    nc.tensor.matmul(
                        misc[:, 3:4], ones_row[:, :], hwrow[:, :],
                        start=True, stop=True,
                    )
                    hwb = lp.tile([128, 1], FP32, tag="hwb")
                    nc.vector.tensor_copy(hwb[:, :], misc[:, 3:4])

                    # softmax over tau (no max-sub: scores are O(0.3))
                    eh = lp.tile([128, 1], cdt, tag="eh")
                    nc.scalar.activation(eh[:, :], sc[:, :], AF.Exp)
                    # Z broadcast to all partitions in one mm: ones128.T @ eh
                    zb = zbp.tile([128, 1], FP32, tag="zb")
                    nc.tensor.matmul(
                        zb[:, :], ones128[:, :], eh[:, :], start=True, stop=True
                    )

                    # u'[o] = sum_tau eh[tau] * ICo[tau, o] (unnormalized)
                    pred = predp.tile([128, OT], FP32, tag="pred")
                    for m in range(OT):
                        nc.tensor.matmul(
                            pred[:, m:m + 1],
                            ICo_sb[:, m * 128:(m + 1) * 128],
                            eh[:, :],
                            start=True, stop=True,
                        )
                    # pred_noWoY_t = u'/Z + (IUo[t-1] + Bo)
                    tm1 = (t - 1) % T
                    if USE_DIVIDE:
                        nc.vector.scalar_tensor_tensor(
                            out_sb[:, t * OT:(t + 1) * OT],
                            pred[:, :],
                            zb[:, 0:1],
                            IUoB_sb[:, tm1 * OT:(tm1 + 1) * OT],
                            ALU.divide, ALU.add,
                        )
                    else:
                        rz = lp.tile([128, 1], FP32, tag="rz")
                        nc.vector.reciprocal(rz[:, :], zb[:, :])
                        nc.vector.scalar_tensor_tensor(
                            out_sb[:, t * OT:(t + 1) * OT],
                            pred[:, :],
                            rz[:, 0:1],
                            IUoB_sb[:, tm1 * OT:(tm1 + 1) * OT],
                            ALU.mult, ALU.add,
                        )
                    hwb_prev = hwb

            # ---- epilogue: add WoY (over t) and transpose to [tau, o] ----
            with (
                tc.tile_pool(name="ep_ps", bufs=2, space="PSUM") as epp,
                tc.tile_pool(name="ep_sb", bufs=2) as eps,
            ):
                outT = pp.tile([128, O], FP32, tag="outT")
                for kt in range(OT):
                    po = epp.tile([128, 128], FP32, tag="po")
                    nc.tensor.transpose(
                        po[:, :],
                        out_sb[:, kt:kt + (T - 1) * OT + 1:OT],
                        Id_sb[:, :],
                    )
                    # += WoY[tau] broadcast along o''
                    nc.tensor.matmul(
                        po[:, :], woyrow[:, :], ones_row[:, :],
                        start=False, stop=True, skip_group_check=True,
                    )
                    nc.vector.tensor_copy(outT[:, kt * 128:(kt + 1) * 128], po[:, :])
                nc.sync.dma_start(out_d[:, :], outT[:, :])

    nc.compile()
    return nc


_NC_CACHE = {}


def _get_nc():
    if "nc" not in _NC_CACHE:
        _NC_CACHE["nc"] = build_nc()
    return _NC_CACHE["nc"]


def make_in_maps(inputs, Wa, Ua, Va, Ba, Wo, Uo, Co, Bo, emb):
    Wa = np.asarray(Wa, np.float32)
    Ua = np.asarray(Ua, np.float32)
    Uo = np.asarray(Uo, np.float32)
    Co = np.asarray(Co, np.float32)
    Va_col = np.ascontiguousarray(
        np.asarray(Va, np.float32)[:, 0].reshape(OT, 128).T
    )
    # fold sigmoid's affine (s = 0.5*tanh + 0.5) into the attention key bias:
    # WaS = s@Wa = 0.5*(tanh_h@Wa) + 0.5*colsum(Wa)
    ba_adj = (
        np.asarray(Ba, np.float64)[0]
        + 0.5 * np.asarray(Wa, np.float64).sum(axis=0)
    ).astype(np.float32)
    Ba_col = np.ascontiguousarray(ba_adj.reshape(OT, 128).T)
    Bo_col = np.ascontiguousarray(
        np.asarray(Bo, np.float32)[0].reshape(OT, 128).T
    )
    ebW = (np.asarray(emb, np.float64) @ np.asarray(Wo, np.float64)).astype(np.float32)
    ebW_col = ebW[:, 0].reshape(OT, 128).T
    oeb = np.ones((128, 2 * OT), dtype=np.float32)
    oeb[:, 1::2] = ebW_col
    oeb = np.ascontiguousarray(oeb)
    Id = np.eye(128, dtype=np.float32)
    shared = dict(
        Wa=Wa, Ua=Ua, Uo=Uo, Co=Co,
        Va_col=Va_col, Ba_col=Ba_col, Bo_col=Bo_col, oeb=oeb, Id=Id,
    )
    return [
        {"x": np.ascontiguousarray(np.asarray(inputs[b], np.float32)), **shared}
        for b in range(B)
    ]


def kernel(inputs, Wa, Ua, Va, Ba, Wo, Uo, Co, Bo, emb):
    nc = _get_nc()
    in_maps = make_in_maps(inputs, Wa, Ua, Va, Ba, Wo, Uo, Co, Bo, emb)
    res = run_bass_kernel_spmd(nc, in_maps, list(range(B)))
    out = np.stack([res.results[b]["out"] for b in range(B)], axis=0)
    return out.astype(np.float32)


if __name__ == "__main__":
    rng = np.random.default_rng(0)
    w = 0.02
    ins = dict(
        inputs=rng.standard_normal((B, T, D), dtype=np.float32),
        Wa=rng.standard_normal((O, O), dtype=np.float32) * w,
        Ua=rng.standard_normal((D, O), dtype=np.float32) * w,
        Va=rng.standard_normal((O, 1), dtype=np.float32) * w,
        Ba=rng.standard_normal((1, O), dtype=np.float32) * w,
        Wo=rng.standard_normal((O, 1), dtype=np.float32) * w,
        Uo=rng.standard_normal((D, O), dtype=np.float32) * w,
        Co=rng.standard_normal((D, O), dtype=np.float32) * w,
        Bo=rng.standard_normal((1, O), dtype=np.float32) * w,
        emb=rng.standard_normal((O, O), dtype=np.float32) * w,
    )
    out = kernel(**ins)
    print(out.shape, out.dtype, np.abs(out).mean())
